# revision 1
# baseline (speedup 1.0000x reference)
"""Minibatch discrimination kernel for 8 trn2 NeuronCores.

reference:
    M = (x @ T).reshape(B, K, D)                       # B=1024, K=50, D=5
    abs_diffs[i,k,j] = sum_d |M[i,k,d] - M[j,k,d]|
    feat[i,k] = sum_j exp(-abs_diffs[i,k,j])
    out = concat([x, feat], axis=1)                    # [1024, 562]

Sharding: rows of x (batch) split across 8 cores, 128 query rows each.
Every core recomputes the full M^T (cheap) so no collectives are needed.

Per core mapping (i = 128 local query rows on partitions, j = 1024 keys on
the free axis):
 - PE broadcasts row c of M^T across 128 partitions with a one-hot matmul
   (one-hot lhsT stationary, 32-row-aligned slice of M^T moving).
 - ScalarE computes |M_i - M_j| = Abs(-psum + bias) with per-partition bias
   M_local[:, c] for 4 of 5 planes; DVE covers the 5th via
   |d| = relu(d) - min(d, 0) (two tensor_scalar ops + a subtract).
 - fp16 tensor_tensor adds accumulate the 5 planes into L1.
 - ScalarE Exp(-L1) with accum_out produces feat[:, k] (row sum fused).
"""

import sys

sys.path.insert(0, "/opt/trn_rl_repo")

from contextlib import ExitStack

import numpy as np

import concourse.bass as bass
import concourse.bacc as bacc
import concourse.tile as tile
from concourse import mybir
from concourse.bass_utils import run_bass_kernel_spmd

B, F = 1024, 512
K, D = 50, 5
C = K * D  # 250 columns of M
NCORES = 8
ROWS = B // NCORES  # 128 query rows per core

f32 = mybir.dt.float32
f16 = mybir.dt.float16

# planes the scalar engine drains; the last plane goes to DVE (relu-min pair)
SCALAR_PLANES = (0, 1, 2, 3)


def _build_program():
    nc = bacc.Bacc("TRN2", target_bir_lowering=False)

    xT = nc.dram_tensor("xT", [F, B], f32, kind="ExternalInput").ap()
    xTloc = nc.dram_tensor("xTloc", [F, ROWS], f32, kind="ExternalInput").ap()
    Tm = nc.dram_tensor("Tm", [F, C], f32, kind="ExternalInput").ap()
    onehot = nc.dram_tensor("onehot", [128, 32 * 128], f16, kind="ExternalInput").ap()
    feat = nc.dram_tensor("feat", [ROWS, K], f32, kind="ExternalOutput").ap()

    with tile.TileContext(nc) as tc, ExitStack() as ctx:
        const_pool = ctx.enter_context(tc.tile_pool(name="const", bufs=1))
        build_psum = ctx.enter_context(tc.tile_pool(name="bpsum", bufs=1, space="PSUM"))
        bc_psum = ctx.enter_context(tc.tile_pool(name="bcpsum", bufs=3, space="PSUM"))
        plane_pool = ctx.enter_context(tc.tile_pool(name="planes", bufs=12))
        tmp_pool = ctx.enter_context(tc.tile_pool(name="tmps", bufs=6))
        scratch_pool = ctx.enter_context(tc.tile_pool(name="scratch", bufs=4))

        # ---- load inputs -------------------------------------------------
        xt_sb = []
        t_sb = []
        xtl_sb = []
        for fc in range(4):
            t = const_pool.tile([128, B], f32, tag=f"xt{fc}")
            nc.sync.dma_start(out=t[:], in_=xT[128 * fc : 128 * (fc + 1), :])
            xt_sb.append(t)
            t2 = const_pool.tile([128, C], f32, tag=f"tm{fc}")
            nc.sync.dma_start(out=t2[:], in_=Tm[128 * fc : 128 * (fc + 1), :])
            t_sb.append(t2)
            t3 = const_pool.tile([128, ROWS], f32, tag=f"xtl{fc}")
            nc.sync.dma_start(out=t3[:], in_=xTloc[128 * fc : 128 * (fc + 1), :])
            xtl_sb.append(t3)
        oh_sb = const_pool.tile([128, 32 * 128], f16, tag="onehot")
        nc.sync.dma_start(out=oh_sb[:], in_=onehot[:, :])

        # PE may carry at most one sync wait per fused matmul (walrus
        # S3_LW limit). Give PE one dummy matmul per DMA-queue sem it will
        # need, so every real matmul below waits on at most one new sem.
        ps_dummy = build_psum.tile([128, 512], f32, tag="bld", name="ps_dummy")
        for dt_tile in (xt_sb[0], xt_sb[1], xt_sb[2], xt_sb[3], oh_sb):
            nc.tensor.matmul(
                out=ps_dummy[:, :],
                lhsT=dt_tile[0:32, 0:128],
                rhs=dt_tile[0:32, 0:512],
                start=True,
                stop=True,
                tile_position=(0, 0),
            )

        # ---- build M^T ([250,1024] as 2 tiles of [128,1024]) -------------
        mt_sb = [
            const_pool.tile([128, B], f16, tag="mt0", name="mt0"),
            const_pool.tile([128, B], f16, tag="mt1", name="mt1"),
        ]
        # zero block 1 first so its 6 pad rows never feed junk into the matmul
        nc.vector.memset(mt_sb[1][:, :], 0.0)
        for blk in range(2):
            cw = 128 if blk == 0 else C - 128  # 128, then 122
            for jh in range(2):
                ps = build_psum.tile([128, 512], f32, tag="bld")
                for fc in range(4):
                    nc.tensor.matmul(
                        out=ps[:cw, :],
                        lhsT=t_sb[fc][:, 128 * blk : 128 * blk + cw],
                        rhs=xt_sb[fc][:, 512 * jh : 512 * (jh + 1)],
                        start=(fc == 0),
                        stop=(fc == 3),
                    )
                nc.scalar.copy(mt_sb[blk][:cw, 512 * jh : 512 * (jh + 1)], ps[:cw, :])

        # ---- build M_local [128, 250] ------------------------------------
        mloc = const_pool.tile([128, C], f32, tag="mloc")
        ps = build_psum.tile([128, 512], f32, tag="bld")
        for fc in range(4):
            nc.tensor.matmul(
                out=ps[:, :C],
                lhsT=xtl_sb[fc][:],
                rhs=t_sb[fc][:],
                start=(fc == 0),
                stop=(fc == 3),
            )
        nc.scalar.copy(mloc[:], ps[:, :C])

        feat_sb = const_pool.tile([128, K], f32, tag="feat")
        zeros16 = const_pool.tile([128, B], f16, tag="zeros16")
        nc.vector.memset(zeros16[:, :], 0.0)

        # ---- main loop over the 50 kernels -------------------------------
        for k in range(K):
            planes = []
            for d in range(D):
                c = 5 * k + d
                blk, r = divmod(c, 128)
                bbase = (r // 32) * 32
                c0 = r % 32
                ps = bc_psum.tile([128, B], f32, tag="bc")
                for jh in range(2):
                    nc.tensor.matmul(
                        out=ps[:, 512 * jh : 512 * (jh + 1)],
                        lhsT=oh_sb[bbase : bbase + 32, 128 * c0 : 128 * (c0 + 1)],
                        rhs=mt_sb[blk][bbase : bbase + 32, 512 * jh : 512 * (jh + 1)],
                        start=True,
                        stop=True,
                        tile_position=(bbase, 0),
                    )
                pl = plane_pool.tile([128, B], f16, tag="plane")
                if d in SCALAR_PLANES:
                    nc.scalar.activation(
                        pl[:],
                        ps[:],
                        mybir.ActivationFunctionType.Abs,
                        bias=mloc[:, c : c + 1],
                        scale=-1.0,
                    )
                else:
                    # |diff| = relu(diff) - min(diff, 0), all walrus-legal ops
                    pa = plane_pool.tile([128, B], f16, tag="pa")
                    nc.vector.tensor_scalar(
                        pa[:], ps[:], mloc[:, c : c + 1], 0.0,
                        op0=mybir.AluOpType.subtract, op1=mybir.AluOpType.max,
                    )
                    pb = plane_pool.tile([128, B], f16, tag="pb")
                    nc.vector.tensor_scalar(
                        pb[:], ps[:], mloc[:, c : c + 1], 0.0,
                        op0=mybir.AluOpType.subtract, op1=mybir.AluOpType.min,
                    )
                    nc.vector.tensor_tensor(
                        out=pl[:], in0=pa[:], in1=pb[:], op=mybir.AluOpType.subtract
                    )
                planes.append(pl)

            t01 = tmp_pool.tile([128, B], f16, tag="t01")
            nc.vector.tensor_tensor(
                out=t01[:], in0=planes[0][:], in1=planes[1][:], op=mybir.AluOpType.add
            )
            t23 = tmp_pool.tile([128, B], f16, tag="t23")
            nc.vector.tensor_tensor(
                out=t23[:], in0=planes[2][:], in1=planes[3][:], op=mybir.AluOpType.add
            )
            t0123 = tmp_pool.tile([128, B], f16, tag="t0123")
            nc.vector.tensor_tensor(
                out=t0123[:], in0=t01[:], in1=t23[:], op=mybir.AluOpType.add
            )
            l1 = tmp_pool.tile([128, B], f16, tag="l1")
            nc.vector.tensor_tensor(
                out=l1[:], in0=t0123[:], in1=planes[4][:], op=mybir.AluOpType.add
            )

            ex = scratch_pool.tile([128, B], f16, tag="ex")
            nc.scalar.activation(
                ex[:],
                l1[:],
                mybir.ActivationFunctionType.Exp,
                bias=0.0,
                scale=-1.0,
                accum_out=feat_sb[:, k : k + 1],
            )

        nc.sync.dma_start(out=feat[:, :], in_=feat_sb[:, :K])

    nc.compile()
    return nc


_program_cache = {}


def _get_program():
    if "nc" not in _program_cache:
        _program_cache["nc"] = _build_program()
    return _program_cache["nc"]


def _make_onehot():
    oh = np.zeros((128, 32 * 128), dtype=np.float16)
    for p in range(128):
        oh[p, (p % 32) * 128 : (p % 32 + 1) * 128] = 1.0
    return oh


def kernel(x: np.ndarray, T: np.ndarray, _trace=False, _trace_kwargs=None):
    x = np.asarray(x, dtype=np.float32)
    T = np.asarray(T, dtype=np.float32)
    nc = _get_program()

    xT_full = np.ascontiguousarray(x.T)  # [512, 1024]
    oh = _make_onehot()
    in_maps = []
    for i in range(NCORES):
        in_maps.append(
            {
                "xT": xT_full,
                "xTloc": np.ascontiguousarray(x.T[:, ROWS * i : ROWS * (i + 1)]),
                "Tm": T,
                "onehot": oh,
            }
        )

    res = run_bass_kernel_spmd(
        nc,
        in_maps,
        core_ids=list(range(NCORES)),
        trace=_trace,
        **(_trace_kwargs or {}),
    )
    feats = np.concatenate([res.results[i]["feat"] for i in range(NCORES)], axis=0)
    out = np.concatenate([x, feats.astype(np.float32)], axis=1)
    if _trace:
        return out, res
    return out

